# revision 1
# baseline (speedup 1.0000x reference)
"""RNN-T joiner (nn_CombinationModel_53154515256115) as a Bass/Tile SPMD kernel
for 8 Trainium2 NeuronCores.

Algorithm
---------
The reference computes, for each valid (b, t, u):
    out[b,t,u] = relu(enc[b,t] @ Wj1_enc + pred[b,u] @ Wj1_pred + bj1) @ Wj2 + bj2
The joint pre-activation factors into a per-(b,t) term A and a per-(b,u) term
Pp, collapsing the first joiner matmul from ~95 GFLOP to ~2 GFLOP. The
remaining dominant work is the [N,640] @ [640,1056] output matmul (bf16 on the
PE) plus the ragged broadcast-add expansion (DVE) and the 272 MB output write.

Sharding (SPMD-uniform)
-----------------------
Core c takes encoder frames t with t % 8 == c from every batch. Every core
then runs an identical program shape: per batch b it owns G[b] = ceil(T_b/8)
frame-groups of (U_b+1) rows each (8134 rows/core; rows of garbage frame-
groups where c + 8g >= T_b are dropped on the host). The tiny prediction
network (328 rows) is computed replicated on every core.
"""

import math
from contextlib import ExitStack

import numpy as np

import concourse.bass as bass
import concourse.mybir as mybir
import concourse.tile as tile
from concourse import bacc
from concourse.masks import make_identity
from concourse.bass import IndirectOffsetOnAxis
from concourse.bass_utils import run_bass_kernel_spmd

F32 = mybir.dt.float32
BF16 = mybir.dt.bfloat16
I32 = mybir.dt.int32
AF = mybir.ActivationFunctionType

# ---------------------------------------------------------------- constants
B, T, U = 8, 300, 40
E, P, J, V = 512, 640, 640, 1056
H, DEMB = 2, 256
ENC_SIZES = [300, 280, 260, 240, 220, 210, 205, 200]
TGT_SIZES = [40, 38, 35, 33, 30, 28, 26, 25]
NCORES = 8
N_FLAT = 64385

G = [(t + NCORES - 1) // NCORES for t in ENC_SIZES]       # groups/core/batch
UB1 = [u + 1 for u in TGT_SIZES]                          # u-extent per batch
RBV = [G[b] * UB1[b] for b in range(B)]                   # valid rows/batch
ROWS = sum(RBV)                                           # 8134 rows/core
GT_TOT = sum(G)                                           # 242 enc frames/core
GT_PAD = 256
OFF_T = [0]
for b in range(B):
    OFF_T.append(OFF_T[-1] + G[b])
OFF_R = [0]
for b in range(B):
    OFF_R.append(OFF_R[-1] + RBV[b])

KJ1_ENC = E // 128            # 4 k-tiles of W_j1 enc part
KJ1_PRED = P // 128           # 5 k-tiles of W_j1 pred part
KJ2 = J // 128                # 5 k-tiles of W_j2
NJ = J // 128                 # 5 partition tiles of the 640-dim feature axis
V_CHUNKS = [(0, 512), (512, 512), (1024, V - 1024)]

_cache = {}


def _build(reps=1):
    nc = bacc.Bacc("TRN2", target_bir_lowering=False, debug=False,
                   num_devices=NCORES)

    enc_sel = nc.dram_tensor("enc_sel", [GT_PAD, E], F32, kind="ExternalInput").ap()
    ctx_idx = nc.dram_tensor("ctx_idx", [128, 4], I32, kind="ExternalInput").ap()
    emb_d = nc.dram_tensor("emb", [V, DEMB], F32, kind="ExternalInput").ap()
    wj1_d = nc.dram_tensor("w_j1", [E + P, J], F32, kind="ExternalInput").ap()
    wj2_d = nc.dram_tensor("w_j2", [J, V], F32, kind="ExternalInput").ap()
    wp1_d = nc.dram_tensor("w_p1", [H * DEMB, P], F32, kind="ExternalInput").ap()
    wp2_d = nc.dram_tensor("w_p2", [P, P], F32, kind="ExternalInput").ap()
    bp1_d = nc.dram_tensor("b_p1", [P], F32, kind="ExternalInput").ap()
    bp2_d = nc.dram_tensor("b_p2", [P], F32, kind="ExternalInput").ap()
    bj1_d = nc.dram_tensor("b_j1", [J], F32, kind="ExternalInput").ap()
    bj2_d = nc.dram_tensor("b_j2", [V], F32, kind="ExternalInput").ap()
    out_d = nc.dram_tensor("out", [ROWS, V], F32, kind="ExternalOutput").ap()

    with tile.TileContext(nc) as tc:
      for _rep in range(reps):
       with ExitStack() as ctx:
        persist = ctx.enter_context(tc.tile_pool(name="persist", bufs=1))
        stage = ctx.enter_context(tc.tile_pool(name="stage", bufs=1))
        wload = ctx.enter_context(tc.tile_pool(name="wload", bufs=3))
        expand = ctx.enter_context(tc.tile_pool(name="expand", bufs=3))
        outp = ctx.enter_context(tc.tile_pool(name="outp", bufs=3))
        ps_small = ctx.enter_context(tc.tile_pool(name="ps_small", bufs=2, space="PSUM"))
        ps_main = ctx.enter_context(tc.tile_pool(name="ps_main", bufs=2, space="PSUM"))

        # ---------------- persistent SBUF state
        wj2_b = [persist.tile([128, V], BF16, tag=f"wj2_{k}", name=f"wj2_{k}") for k in range(KJ2)]
        at_t = [persist.tile([128, GT_TOT], F32, tag=f"at_{j}", name=f"at_{j}") for j in range(NJ)]
        pp_t = [persist.tile([128, 41 * B], F32, tag=f"pp_{j}", name=f"pp_{j}") for j in range(NJ)]
        bt_t = persist.tile([128, V], F32, tag="bt", name="bt")
        ht_t = [persist.tile([128, ROWS], BF16, tag=f"ht_{j}", name=f"ht_{j}")
                for j in range(NJ)]

        # HWDGE fp32 load + ACT cast to bf16 (keeps the SWDGE/Pool queue free
        # for the embedding gathers)
        def cast_load(dst, src_ap, width):
            wl = wload.tile([128, V], F32, tag="wl", name="wl")
            nc.sync.dma_start(out=wl[:, 0:width], in_=src_ap)
            nc.vector.tensor_copy(dst, wl[:, 0:width])

        # ---------------- embedding gather (Pool queue, first in line)
        idx_t = stage.tile([128, 4], I32, tag="idx")
        nc.sync.dma_start(out=idx_t[:], in_=ctx_idx)
        g_b16 = [stage.tile([128, DEMB], BF16, tag=f"gb_{c}", name=f"gb_{c}") for c in range(4)]
        for c in range(4):
            nc.gpsimd.indirect_dma_start(
                out=g_b16[c][:], out_offset=None, in_=emb_d,
                in_offset=IndirectOffsetOnAxis(ap=idx_t[:, c:c + 1], axis=0))
        ident = stage.tile([128, 128], BF16, tag="ident", name="ident")
        make_identity(nc, ident[:])
        gt_h = [stage.tile([128, 512], BF16, tag=f"gt_{h}", name=f"gt_{h}") for h in range(2)]
        for c in range(4):
            for h in range(2):
                pst = ps_small.tile([128, 128], BF16, tag="ps_s", name="ps_tr")
                nc.tensor.transpose(pst[:], g_b16[c][:, h * 128:(h + 1) * 128],
                                    ident[:])
                nc.vector.tensor_copy(gt_h[h][:, c * 128:(c + 1) * 128], pst[:])

        # ---------------- encoder load + cast + transpose
        enc_bf = [stage.tile([128, E], BF16, tag=f"encb_{p}", name=f"encb_{p}") for p in range(2)]
        for p in range(2):
            cast_load(enc_bf[p][:], enc_sel[p * 128:(p + 1) * 128, :], E)
        encT = [stage.tile([128, GT_PAD], BF16, tag=f"encT_{f}", name=f"encT_{f}")
                for f in range(KJ1_ENC)]
        for f in range(KJ1_ENC):
            for p in range(2):
                pst = ps_small.tile([128, 128], BF16, tag="ps_s", name="ps_tr")
                nc.tensor.transpose(pst[:], enc_bf[p][:, f * 128:(f + 1) * 128],
                                    ident[:])
                nc.vector.tensor_copy(encT[f][:, p * 128:(p + 1) * 128], pst[:])

        # ---------------- weights and biases
        wp1_b = [stage.tile([128, P], BF16, tag=f"wp1_{k}", name=f"wp1_{k}") for k in range(4)]
        for k in range(4):
            cast_load(wp1_b[k][:], wp1_d[k * 128:(k + 1) * 128, :], P)
        wp2_b = [stage.tile([128, P], BF16, tag=f"wp2_{k}", name=f"wp2_{k}") for k in range(5)]
        wj1_b = [stage.tile([128, J], BF16, tag=f"wj1_{k}", name=f"wj1_{k}")
                 for k in range(KJ1_ENC + KJ1_PRED)]
        # AT needs the enc half of W_j1 early
        for k in range(KJ1_ENC):
            cast_load(wj1_b[k][:], wj1_d[k * 128:(k + 1) * 128, :], J)

        bp1_t = stage.tile([128, 5], F32, tag="bp1")
        bp2_t = stage.tile([128, 5], F32, tag="bp2")
        bj1_t = stage.tile([128, 5], F32, tag="bj1")
        nc.sync.dma_start(out=bp1_t[:], in_=bp1_d.rearrange("(a p) -> p a", p=128))
        nc.sync.dma_start(out=bp2_t[:], in_=bp2_d.rearrange("(a p) -> p a", p=128))
        nc.sync.dma_start(out=bj1_t[:], in_=bj1_d.rearrange("(a p) -> p a", p=128))
        nc.sync.dma_start(out=bt_t[:], in_=bj2_d[None, :].to_broadcast([128, V]))

        # ---------------- prediction network (replicated, 328 rows)
        def e_tile(h, o):
            return gt_h[h][:].rearrange("p (b k) -> p b k", b=8)[:, :, o:o + 41]

        e_ktiles = [e_tile(0, 1), e_tile(1, 1), e_tile(0, 0), e_tile(1, 0)]
        h1_t = [stage.tile([128, 41 * B], BF16, tag=f"h1_{j}", name=f"h1_{j}") for j in range(5)]
        for j in range(5):
            ps = ps_small.tile([128, 41 * B], F32, tag="ps_s", name="ps_h1")
            for k in range(4):
                nc.tensor.matmul(out=ps[:].rearrange("p (b u) -> p b u", b=8),
                                 lhsT=wp1_b[k][:, j * 128:(j + 1) * 128],
                                 rhs=e_ktiles[k],
                                 start=(k == 0), stop=(k == 3))
            nc.scalar.activation(h1_t[j][:], ps[:], AF.Tanh,
                                 bias=bp1_t[:, j:j + 1], scale=1.0)
        for k in range(5):
            cast_load(wp2_b[k][:], wp2_d[k * 128:(k + 1) * 128, :], P)
        pred_t = [stage.tile([128, 41 * B], BF16, tag=f"pred_{j}", name=f"pred_{j}")
                  for j in range(5)]
        for j in range(5):
            ps = ps_small.tile([128, 41 * B], F32, tag="ps_s", name="ps_p2")
            for k in range(5):
                nc.tensor.matmul(out=ps[:],
                                 lhsT=wp2_b[k][:, j * 128:(j + 1) * 128],
                                 rhs=h1_t[k][:],
                                 start=(k == 0), stop=(k == 4))
            nc.scalar.activation(pred_t[j][:], ps[:], AF.Tanh,
                                 bias=bp2_t[:, j:j + 1], scale=1.0)
        for k in range(KJ1_ENC, KJ1_ENC + KJ1_PRED):
            cast_load(wj1_b[k][:], wj1_d[k * 128:(k + 1) * 128, :], J)
        for j in range(5):
            ps = ps_small.tile([128, 41 * B], F32, tag="ps_s", name="ps_pp")
            for k in range(5):
                nc.tensor.matmul(out=ps[:],
                                 lhsT=wj1_b[KJ1_ENC + k][:, j * 128:(j + 1) * 128],
                                 rhs=pred_t[k][:],
                                 start=(k == 0), stop=(k == 4))
            nc.vector.tensor_copy(pp_t[j][:], ps[:])

        # ---------------- A = enc @ Wj1_enc + bj1   (transposed layout)
        for j in range(5):
            ps = ps_small.tile([128, GT_TOT], F32, tag="ps_s", name="ps_at")
            for k in range(KJ1_ENC):
                nc.tensor.matmul(out=ps[:],
                                 lhsT=wj1_b[k][:, j * 128:(j + 1) * 128],
                                 rhs=encT[k][:, 0:GT_TOT],
                                 start=(k == 0), stop=(k == KJ1_ENC - 1))
            nc.scalar.activation(at_t[j][:], ps[:], AF.Identity,
                                 bias=bj1_t[:, j:j + 1], scale=1.0)

        for k in range(KJ2):
            wl = wload.tile([128, V], F32, tag="wl", name="wl")
            nc.sync.dma_start(out=wl[:], in_=wj2_d[k * 128:(k + 1) * 128, :])
            nc.vector.tensor_copy(wj2_b[k][:], wl[:])

        # ---------------- ragged expansion + main loop, emission-interleaved
        # so the DVE stream orders each batch's psum evacuations before the
        # next batch's expansion adds.
        def emit_expand(b, j):
            g, u1, rv = G[b], UB1[b], RBV[b]
            tmp = expand.tile([128, max(RBV)], BF16, tag="tmp", name="tmp")
            nc.gpsimd.tensor_tensor(
                out=tmp[:, 0:rv].rearrange("p (g u) -> p g u", g=g),
                in0=at_t[j][:, OFF_T[b]:OFF_T[b] + g][:, :, None]
                    .to_broadcast([128, g, u1]),
                in1=pp_t[j][:, b * 41: b * 41 + u1][:, None, :]
                    .to_broadcast([128, g, u1]),
                op=mybir.AluOpType.add)
            nc.scalar.activation(ht_t[j][:, OFF_R[b]:OFF_R[b] + rv], tmp[:, 0:rv],
                                 AF.Relu, scale=1.0)

        def emit_main_tile(rt):
            m = min(128, ROWS - rt * 128)
            ps = ps_main.tile([128, V], F32, tag="ps_out", name="ps_out")
            for k in range(KJ2):
                for (c0, cn) in V_CHUNKS:
                    nc.tensor.matmul(
                        out=ps[0:m, c0:c0 + cn],
                        lhsT=ht_t[k][:, rt * 128: rt * 128 + m],
                        rhs=wj2_b[k][:, c0:c0 + cn],
                        start=(k == 0), stop=(k == KJ2 - 1))
            osb = outp.tile([128, V], F32, tag="osb", name="osb")
            nc.vector.tensor_tensor(out=osb[0:m], in0=ps[0:m], in1=bt_t[0:m],
                                    op=mybir.AluOpType.add)
            nc.sync.dma_start(
                out=out_d[rt * 128: rt * 128 + m, :],
                in_=osb[0:m])

        NTILES = (ROWS + 127) // 128
        exp_q = [(b, j) for b in range(B) for j in range(NJ)]
        # hard deadline: a batch's expansion must precede the tiles that read
        # it; soft window: trickle one op per tile slot well ahead so the DVE
        # stream interleaves expansion adds with psum evacuations instead of
        # bursting 5 adds at each batch boundary.
        for rt in range(NTILES):
            while exp_q and OFF_R[exp_q[0][0]] < (rt + 2) * 128:
                emit_expand(*exp_q.pop(0))
            if exp_q and OFF_R[exp_q[0][0]] < (rt + 14) * 128:
                emit_expand(*exp_q.pop(0))
            emit_main_tile(rt)

    nc.compile()
    return nc


def _host_inputs(inputs):
    """Build per-core in_maps from the full inputs."""
    enc = np.ascontiguousarray(np.asarray(inputs["encoder_states"], dtype=np.float32))
    targets = np.asarray(inputs["targets"]).astype(np.int64)
    emb = np.ascontiguousarray(np.asarray(inputs["emb"], dtype=np.float32))

    ext = np.zeros((B, 64), np.int64)
    ext[:, 2:2 + U] = targets
    idx = np.zeros((128, 4), np.int32)
    for s in range(4):
        idx[0:64, s] = ext[2 * s]
        idx[64:128, s] = ext[2 * s + 1]

    common = {
        "ctx_idx": idx,
        "emb": emb,
        "w_j1": np.asarray(inputs["W_j1"], dtype=np.float32),
        "w_j2": np.asarray(inputs["W_j2"], dtype=np.float32),
        "w_p1": np.asarray(inputs["W_pred1"], dtype=np.float32),
        "w_p2": np.asarray(inputs["W_pred2"], dtype=np.float32),
        "b_p1": np.asarray(inputs["b_pred1"], dtype=np.float32),
        "b_p2": np.asarray(inputs["b_pred2"], dtype=np.float32),
        "b_j1": np.asarray(inputs["b_j1"], dtype=np.float32),
        "b_j2": np.asarray(inputs["b_j2"], dtype=np.float32),
    }
    in_maps = []
    for c in range(NCORES):
        enc_sel = np.zeros((GT_PAD, E), np.float32)
        for b in range(B):
            ts = c + NCORES * np.arange(G[b])
            valid = ts < ENC_SIZES[b]
            rows = np.where(valid)[0]
            enc_sel[OFF_T[b] + rows] = enc[b, ts[valid]]
        in_maps.append({"enc_sel": enc_sel, **common})
    return in_maps


def _gather_output(core_outs, inputs):
    fb = np.asarray(inputs["flat_b"]).astype(np.int64)
    ft = np.asarray(inputs["flat_t"]).astype(np.int64)
    fu = np.asarray(inputs["flat_u"]).astype(np.int64)
    ub1 = np.asarray(UB1, np.int64)
    off_r = np.asarray(OFF_R[:B], np.int64)
    core = ft % NCORES
    local = off_r[fb] + (ft // NCORES) * ub1[fb] + fu
    out = np.empty((fb.shape[0], V), np.float32)
    for c in range(NCORES):
        m = core == c
        out[m] = core_outs[c][local[m]]
    return out


def kernel(**inputs) -> np.ndarray:
    if "nc" not in _cache:
        _cache["nc"] = _build()
    nc = _cache["nc"]
    in_maps = _host_inputs(inputs)
    res = run_bass_kernel_spmd(nc, in_maps, list(range(NCORES))).results
    core_outs = [res[c]["out"] for c in range(NCORES)]
    return _gather_output(core_outs, inputs)



# revision 22
# speedup vs baseline: 1.2489x; 1.2489x over previous
"""RNN-T joiner (nn_CombinationModel_53154515256115) as a Bass/Tile SPMD kernel
for 8 Trainium2 NeuronCores.

Algorithm
---------
The reference computes, for each valid (b, t, u):
    out[b,t,u] = relu(enc[b,t] @ Wj1_enc + pred[b,u] @ Wj1_pred + bj1) @ Wj2 + bj2
The joint pre-activation factors into a per-(b,t) term A and a per-(b,u) term
Pp, collapsing the first joiner matmul from ~95 GFLOP to ~2 GFLOP. The
remaining dominant work is the [N,640] @ [640,1056] output matmul plus the
ragged broadcast-add expansion and the output write.

The output matmul runs in fp8 (e4m3) with DoubleRow perf mode (two
contraction k-tiles per instruction at 0.5 cycles/row) and full
error-feedback so precision stays at bf16 level:
    psum = h8@W8 + dh8@W8 + h8@dW8  (+ 2^16 * b_j2)
with h8 = e4m3(64*h), dh8 = e4m3(64*h - h8), W8 = e4m3(1024*W_j2),
dW8 = e4m3(1024*W_j2 - W8). The psum is therefore 2^16 * out; it is written
as fp16 and rescaled by 2^-16 on the host (exact power of two).
Per 128-row tile that is 8 DoubleRow matmuls (vs 15 bf16 k-tile equivalents).

Weights arrive pre-cast (bf16 / e4m3) and pre-transposed from the host (one
batched DMA per tensor), and the prediction-network embedding gather runs on
the host, so the device does no fp32 weight traffic, no embedding gather and
no PE transposes. The elementwise expansion work (broadcast add, relu/cast,
residual, psum evacuation) is statically load-balanced across Pool, Act and
DVE.

Sharding (SPMD-uniform)
-----------------------
Core c takes encoder frames t with t % 8 == c from every batch. Every core
then runs an identical program shape: per batch b it owns G[b] = ceil(T_b/8)
frame-groups of (U_b+1) rows each (8134 rows/core; rows of garbage frame-
groups where c + 8g >= T_b are dropped on the host). The tiny prediction
network (264 packed rows) is computed replicated on every core.
"""

from contextlib import ExitStack

import numpy as np
import ml_dtypes

import concourse.bass as bass
import concourse.mybir as mybir
import concourse.tile as tile
from concourse import bacc
from concourse.bass_utils import run_bass_kernel_spmd

F32 = mybir.dt.float32
F16 = mybir.dt.float16
BF16 = mybir.dt.bfloat16
F8E4 = mybir.dt.float8e4
AF = mybir.ActivationFunctionType
ALU = mybir.AluOpType
DR = mybir.MatmulPerfMode.DoubleRow

NP_BF16 = ml_dtypes.bfloat16
NP_E4 = ml_dtypes.float8_e4m3

# ---------------------------------------------------------------- constants
B, T, U = 8, 300, 40
E, P, J, V = 512, 640, 640, 1056
H, DEMB = 2, 256
ENC_SIZES = [300, 280, 260, 240, 220, 210, 205, 200]
TGT_SIZES = [40, 38, 35, 33, 30, 28, 26, 25]
NCORES = 8

G = [(t + NCORES - 1) // NCORES for t in ENC_SIZES]       # groups/core/batch
UB1 = [u + 1 for u in TGT_SIZES]                          # u-extent per batch
RBV = [G[b] * UB1[b] for b in range(B)]                   # valid rows/batch
ROWS = sum(RBV)                                           # 8134 rows/core
GT_TOT = sum(G)                                           # 242 enc frames/core
OFF_T = [0]
for b in range(B):
    OFF_T.append(OFF_T[-1] + G[b])
OFF_R = [0]
for b in range(B):
    OFF_R.append(OFF_R[-1] + RBV[b])
UOFF = [0]
for b in range(B):
    UOFF.append(UOFF[-1] + UB1[b])
UTOT = UOFF[-1]               # 263 valid prediction rows
UPAD = 264
# HD plane stride: DoubleRow Ldweights requires the dual-row step to be a
# multiple of 16 elements (s3_lw_dual_fp8_restrictions)
RPAD = (ROWS + 15) // 16 * 16

NJ = J // 128                 # 5 partition tiles of the 640-dim feature axis
V_CHUNKS = [(0, 512), (512, 512), (1024, V - 1024)]
NTILES = (ROWS + 127) // 128

SH = 64.0                     # h-side fp8 scale
SW = 1024.0                   # W-side fp8 scale  (psum = 2^16 * out)
OUT_SCALE = 1.0 / (SH * SW)

# expansion work units: (b, g0, g1); first batches split so the main loop's
# first tiles are not gated on one huge Pool op per plane
_EXP_SPLITS = {0: 3, 1: 2}
EXP_CHUNKS = []
for b in range(B):
    ns = _EXP_SPLITS.get(b, 1)
    bounds = [round(i * G[b] / ns) for i in range(ns + 1)]
    for i in range(ns):
        EXP_CHUNKS.append((b, bounds[i], bounds[i + 1]))

PS_SMALL_BUFS = 2

# ------------- engine-split knobs (tuned against the instruction cost model)
TRICKLE = 1
DEADLINE = 2
HORIZON = 14
ADD_DVE_EVERY = 4      # every n-th expansion add runs on DVE instead of Pool
H8_POOL_EVERY = 10**9  # disabled: Pool is the critical engine
# evac engine rotation: A=Act, D=DVE, P=Pool
EV_PAT = ("A", "D", "A", "D", "A", "A", "D", "A",
          "A", "D", "A", "A", "A", "D", "A", "D")

_cache = {}


def _build(zero_bj2=True):
    nc = bacc.Bacc("TRN2", target_bir_lowering=False, debug=False,
                   num_devices=NCORES)

    # one dram tensor per logical input, pre-laid-out as [128, ...] so each
    # loads with a single wide DMA
    encT_d = nc.dram_tensor("encT", [128, 4 * 256], BF16, kind="ExternalInput").ap()
    eT_d = nc.dram_tensor("eT", [128, 4 * UPAD], BF16, kind="ExternalInput").ap()
    wp1_d = nc.dram_tensor("w_p1", [128, 4 * P], BF16, kind="ExternalInput").ap()
    wp2_d = nc.dram_tensor("w_p2", [128, 5 * P], BF16, kind="ExternalInput").ap()
    wj1e_d = nc.dram_tensor("w_j1e", [128, 4 * J], BF16, kind="ExternalInput").ap()
    wj1p_d = nc.dram_tensor("w_j1p", [128, 5 * J], BF16, kind="ExternalInput").ap()
    wj2hi_d = nc.dram_tensor("w_j2hi", [128, 5 * V], F8E4, kind="ExternalInput").ap()
    wj2lo_d = nc.dram_tensor("w_j2lo", [128, 3 * 2 * V], F8E4,
                             kind="ExternalInput").ap()
    bias_d = nc.dram_tensor("biases", [128, 15], F32, kind="ExternalInput").ap()
    bj2_d = nc.dram_tensor("b_j2", [V], F32, kind="ExternalInput").ap()
    out_d = nc.dram_tensor("out", [ROWS, V], F16, kind="ExternalOutput").ap()

    R = RPAD
    MAXCH = max((g1 - g0) * UB1[b] for (b, g0, g1) in EXP_CHUNKS)

    with tile.TileContext(nc) as tc:
      with ExitStack() as ctx:
        persist = ctx.enter_context(tc.tile_pool(name="persist", bufs=1))
        stage = ctx.enter_context(tc.tile_pool(name="stage", bufs=1))
        tmpp = ctx.enter_context(tc.tile_pool(name="tmpp", bufs=3))
        hfp = ctx.enter_context(tc.tile_pool(name="hfp", bufs=3))
        outp = ctx.enter_context(tc.tile_pool(name="outp", bufs=3))
        ps_small = ctx.enter_context(tc.tile_pool(name="ps_small", bufs=PS_SMALL_BUFS, space="PSUM"))
        ps_main = ctx.enter_context(tc.tile_pool(name="ps_main", bufs=2, space="PSUM"))

        # ---------------- persistent SBUF state
        # HD planes (free dim, ROWS wide each): h0,d0,h1,d1,...,h4,d4
        HD = persist.tile([128, 10 * R], F8E4, tag="hd", name="hd")
        wj2hi_b = persist.tile([128, 5 * V], F8E4, tag="wj2h", name="wj2h")
        wj2lo_b = persist.tile([128, 3 * 2 * V], F8E4, tag="wj2l", name="wj2l")
        at_t = [persist.tile([128, GT_TOT], BF16, tag=f"at_{j}", name=f"at_{j}")
                for j in range(NJ)]
        pp_t = [persist.tile([128, UPAD], BF16, tag=f"pp_{j}", name=f"pp_{j}")
                for j in range(NJ)]

        # ---------------- input loads: one DMA per tensor, two queues,
        # ordered by first use
        eT_b = stage.tile([128, 4 * UPAD], BF16, tag="eT", name="eT")
        wp1_b = stage.tile([128, 4 * P], BF16, tag="wp1", name="wp1")
        wp2_b = stage.tile([128, 5 * P], BF16, tag="wp2", name="wp2")
        encT_b = stage.tile([128, 4 * 256], BF16, tag="encT", name="encT")
        wj1e_b = stage.tile([128, 4 * J], BF16, tag="wj1e", name="wj1e")
        wj1p_b = stage.tile([128, 5 * J], BF16, tag="wj1p", name="wj1p")
        bias_t = stage.tile([128, 15], F32, tag="bias")

        # first two tensors split in half so stage1's k=0/1 matmuls can
        # start before the full transfers land
        nc.sync.dma_start(out=eT_b[:, 0:2 * UPAD], in_=eT_d[:, 0:2 * UPAD])
        nc.scalar.dma_start(out=wp1_b[:, 0:2 * P], in_=wp1_d[:, 0:2 * P])
        nc.sync.dma_start(out=eT_b[:, 2 * UPAD:], in_=eT_d[:, 2 * UPAD:])
        nc.scalar.dma_start(out=wp1_b[:, 2 * P:], in_=wp1_d[:, 2 * P:])
        nc.scalar.dma_start(out=bias_t[:], in_=bias_d)
        nc.sync.dma_start(out=encT_b[:], in_=encT_d)
        nc.scalar.dma_start(out=wj1e_b[:], in_=wj1e_d)
        nc.sync.dma_start(out=wp2_b[:], in_=wp2_d)
        nc.scalar.dma_start(out=wj1p_b[:], in_=wj1p_d)
        nc.sync.dma_start(out=wj2hi_b[:], in_=wj2hi_d)
        nc.sync.dma_start(out=wj2lo_b[:], in_=wj2lo_d)
        bp1_t = bias_t[:, 0:5]
        bp2_t = bias_t[:, 5:10]
        bj1_t = bias_t[:, 10:15]

        bt_t = None
        if not zero_bj2:
            bt_t = persist.tile([128, V], F32, tag="bt", name="bt")
            nc.sync.dma_start(out=bt_t[:], in_=bj2_d[None, :].to_broadcast([128, V]))

        # ---------------- prediction network (replicated, 264 packed rows)
        h1_t = [stage.tile([128, UPAD], BF16, tag=f"h1_{j}", name=f"h1_{j}") for j in range(5)]
        pred_t = [stage.tile([128, UPAD], BF16, tag=f"pred_{j}", name=f"pred_{j}")
                  for j in range(5)]

        def a_block(j):
            ps = ps_small.tile([128, UPAD], F32, tag="ps_s", name="ps_at")
            for k in range(4):
                nc.tensor.matmul(out=ps[:, 0:GT_TOT],
                                 lhsT=wj1e_b[:, k * J + j * 128:k * J + (j + 1) * 128],
                                 rhs=encT_b[:, k * 256:k * 256 + GT_TOT],
                                 start=(k == 0), stop=(k == 3))
            nc.scalar.activation(at_t[j][:], ps[:, 0:GT_TOT], AF.Identity,
                                 bias=bj1_t[:, j:j + 1], scale=1.0)

        # stage1 interleaved with the A blocks (A only needs DMAs, so it
        # fills PE gaps while the Act engine runs the stage tanhs)
        for j in range(5):
            ps = ps_small.tile([128, UPAD], F32, tag="ps_s", name="ps_h1")
            for k in range(4):
                nc.tensor.matmul(out=ps[:],
                                 lhsT=wp1_b[:, k * P + j * 128:k * P + (j + 1) * 128],
                                 rhs=eT_b[:, k * UPAD:(k + 1) * UPAD],
                                 start=(k == 0), stop=(k == 3))
            nc.scalar.activation(h1_t[j][:], ps[:], AF.Tanh,
                                 bias=bp1_t[:, j:j + 1], scale=1.0)
            a_block(j)
        for j in range(5):
            ps = ps_small.tile([128, UPAD], F32, tag="ps_s", name="ps_p2")
            for k in range(5):
                nc.tensor.matmul(out=ps[:],
                                 lhsT=wp2_b[:, k * P + j * 128:k * P + (j + 1) * 128],
                                 rhs=h1_t[k][:],
                                 start=(k == 0), stop=(k == 4))
            nc.scalar.activation(pred_t[j][:], ps[:], AF.Tanh,
                                 bias=bp2_t[:, j:j + 1], scale=1.0)

        def s3_block(j):
            ps = ps_small.tile([128, UPAD], F32, tag="ps_s", name="ps_pp")
            for k in range(5):
                nc.tensor.matmul(out=ps[:],
                                 lhsT=wj1p_b[:, k * J + j * 128:k * J + (j + 1) * 128],
                                 rhs=pred_t[k][:],
                                 start=(k == 0), stop=(k == 4))
            nc.vector.tensor_copy(pp_t[j][:], ps[:])

        # ---------------- ragged expansion into fp8 h8/dh8 planes
        exp_ctr = [0, 0]        # add-counter, h8-counter

        def emit_expand(ci, j):
            b, g0, g1 = EXP_CHUNKS[ci]
            gg, u1 = g1 - g0, UB1[b]
            rv = gg * u1
            o = OFF_R[b] + g0 * u1
            tmp = tmpp.tile([128, MAXCH], BF16, tag="tmp", name="tmp")
            # tmp = at[t] + pp[u]   (Pool, every n-th on DVE)
            exp_ctr[0] += 1
            eng = nc.vector if exp_ctr[0] % ADD_DVE_EVERY == 0 else nc.gpsimd
            eng.tensor_tensor(
                out=tmp[:, 0:rv].rearrange("p (g u) -> p g u", g=gg),
                in0=at_t[j][:, OFF_T[b] + g0:OFF_T[b] + g1][:, :, None]
                    .to_broadcast([128, gg, u1]),
                in1=pp_t[j][:, UOFF[b]:UOFF[b] + u1][:, None, :]
                    .to_broadcast([128, gg, u1]),
                op=ALU.add)
            # h8 = e4m3(relu(64*tmp))   (Act, every n-th on Pool)
            exp_ctr[1] += 1
            h8_out = HD[:, (2 * j) * R + o:(2 * j) * R + o + rv]
            if exp_ctr[1] % H8_POOL_EVERY == 0:
                nc.gpsimd.tensor_scalar(out=h8_out, in0=tmp[:, 0:rv],
                                        scalar1=SH, scalar2=0.0,
                                        op0=ALU.mult, op1=ALU.max)
            else:
                nc.scalar.activation(h8_out, tmp[:, 0:rv], AF.Relu, scale=SH)
            # hf = max(64*tmp, 0) in bf16   (DVE, 4x mode)
            hf = hfp.tile([128, MAXCH], BF16, tag="hf", name="hf")
            nc.vector.tensor_scalar(out=hf[:, 0:rv], in0=tmp[:, 0:rv],
                                    scalar1=SH, scalar2=0.0,
                                    op0=ALU.mult, op1=ALU.max)
            # dh8 = e4m3(hf - h8)   (DVE)
            nc.vector.tensor_tensor(
                out=HD[:, (2 * j + 1) * R + o:(2 * j + 1) * R + o + rv],
                in0=hf[:, 0:rv],
                in1=h8_out,
                op=ALU.subtract)

        # s3 + pp evac per j, immediately followed by that j's first-chunk
        # expansion so the b=0 pipeline fills as early as possible
        for j in range(5):
            s3_block(j)
            emit_expand(0, j)

        # ---------------- main loop
        def emit_main_tile(rt):
            m = min(128, ROWS - rt * 128)
            r0 = rt * 128
            ps = ps_main.tile([128, V], F32, tag="ps_out", name="ps_out")

            def lhs_pair(plane, stride2):
                # [128, 2, m] view of HD planes (plane, plane+stride2)
                seg = HD[:, plane * R:(plane + 2 * stride2) * R]
                if stride2 == 1:
                    v = seg.rearrange("p (two r) -> p two r", two=2)
                else:
                    v = seg.rearrange("p (two x r) -> p two x r", two=2, x=2)[:, :, 0]
                return v[:, :, r0:r0 + m]

            mms = []
            for j in range(5):      # (h8_j | dh8_j) x (W8_j | W8_j)
                mms.append((lhs_pair(2 * j, 1), None, j))
            mms.append((lhs_pair(0, 2), 0, None))   # (h8_0|h8_1) x (dW0|dW1)
            mms.append((lhs_pair(4, 2), 1, None))   # (h8_2|h8_3) x (dW2|dW3)
            mms.append((lhs_pair(8, 1), 2, None))   # (h8_4|dh8_4) x (dW4|Z)

            for i, (lhsT, q, j) in enumerate(mms):
                for (c0, cn) in V_CHUNKS:
                    if q is None:
                        rhs = wj2hi_b[:, None, j * V + c0:j * V + c0 + cn] \
                            .to_broadcast([128, 2, cn])
                    else:
                        rhs = wj2lo_b[:, 2 * q * V:2 * (q + 1) * V].rearrange(
                            "p (two v) -> p two v", two=2)[:, :, c0:c0 + cn]
                    nc.tensor.matmul(out=ps[0:m, c0:c0 + cn],
                                     lhsT=lhsT, rhs=rhs,
                                     start=(i == 0), stop=(i == 7),
                                     perf_mode=DR)

            osb = outp.tile([128, V], F16, tag="osb", name="osb")
            if zero_bj2:
                ev = EV_PAT[rt % len(EV_PAT)]
                if ev == "A":
                    nc.scalar.activation(osb[0:m], ps[0:m], AF.Copy)
                elif ev == "D":
                    nc.vector.tensor_copy(osb[0:m], ps[0:m])
                else:
                    nc.gpsimd.tensor_copy(osb[0:m], ps[0:m])
            else:
                nc.vector.tensor_tensor(out=osb[0:m], in0=ps[0:m],
                                        in1=bt_t[0:m], op=ALU.add)
            nc.sync.dma_start(out=out_d[r0:r0 + m, :], in_=osb[0:m])

        exp_q = [(ci, j) for ci in range(1, len(EXP_CHUNKS)) for j in range(NJ)]

        def chunk_start_row(ci):
            b, g0, g1 = EXP_CHUNKS[ci]
            return OFF_R[b] + g0 * UB1[b]

        for rt in range(NTILES):
            while exp_q and chunk_start_row(exp_q[0][0]) < (rt + DEADLINE) * 128:
                emit_expand(*exp_q.pop(0))
            for _ in range(TRICKLE):
                if exp_q and chunk_start_row(exp_q[0][0]) < (rt + HORIZON) * 128:
                    emit_expand(*exp_q.pop(0))
            emit_main_tile(rt)

    nc.compile()
    return nc


def _kp(w, kt, cols):
    """[kt*128, cols] -> [128, kt*cols] (k-tile-major columns)."""
    return np.ascontiguousarray(
        w.reshape(kt, 128, cols).transpose(1, 0, 2).reshape(128, kt * cols))


def _host_inputs(inputs):
    """Build per-core in_maps from the full inputs (cast + transpose + gather
    staging only; all model compute stays on device)."""
    enc = np.ascontiguousarray(np.asarray(inputs["encoder_states"], dtype=np.float32))
    targets = np.asarray(inputs["targets"]).astype(np.int64)
    emb = np.asarray(inputs["emb"], dtype=np.float32)
    wj1 = np.asarray(inputs["W_j1"], dtype=np.float32)
    wj2 = np.asarray(inputs["W_j2"], dtype=np.float32)
    bj2 = np.asarray(inputs["b_j2"], dtype=np.float32)

    # prediction-network input e: H-gram context embeddings, padding_idx=0,
    # packed to the 263 valid (b, u) rows (+1 pad row)
    ext = np.zeros((B, U + H), np.int64)
    ext[:, H:] = targets
    ctx = np.stack([ext[:, H - 1 - i: U + H - i] for i in range(H)], axis=-1)
    e = emb[ctx] * (ctx != 0)[..., None]                 # [B, U+1, H, DEMB]
    e = e.reshape(B, U + 1, H * DEMB)
    e_sel = np.zeros((UPAD, H * DEMB), np.float32)
    for b in range(B):
        e_sel[UOFF[b]:UOFF[b] + UB1[b]] = e[b, :UB1[b]]
    eT = np.ascontiguousarray(e_sel.T).astype(NP_BF16)   # [512, 264]

    # fp8 split of W_j2 (scaled by SW) + zero pair-slot for the bias trick
    w_hi = (SW * wj2).astype(NP_E4)
    w_lo = (SW * wj2 - w_hi.astype(np.float32)).astype(NP_E4)
    zero_bj2 = not np.any(bj2)
    wj2lo = np.zeros((3, 128, 2, V), NP_E4)
    wj2lo[0, :, 0] = w_lo[0:128]
    wj2lo[0, :, 1] = w_lo[128:256]
    wj2lo[1, :, 0] = w_lo[256:384]
    wj2lo[1, :, 1] = w_lo[384:512]
    wj2lo[2, :, 0] = w_lo[512:640]
    # pair slot 2/plane 1 multiplies dh8_4 -> keep zero (bias added via bt
    # tile when bj2 != 0)

    biases = np.zeros((128, 15), np.float32)
    biases[:, 0:5] = np.asarray(inputs["b_pred1"], np.float32).reshape(5, 128).T
    biases[:, 5:10] = np.asarray(inputs["b_pred2"], np.float32).reshape(5, 128).T
    biases[:, 10:15] = np.asarray(inputs["b_j1"], np.float32).reshape(5, 128).T

    common = {
        "eT": _kp(eT, 4, UPAD),
        "w_p1": _kp(np.asarray(inputs["W_pred1"], np.float32).astype(NP_BF16), 4, P),
        "w_p2": _kp(np.asarray(inputs["W_pred2"], np.float32).astype(NP_BF16), 5, P),
        "w_j1e": _kp(wj1[:E].astype(NP_BF16), 4, J),
        "w_j1p": _kp(wj1[E:].astype(NP_BF16), 5, J),
        "w_j2hi": _kp(w_hi, 5, V),
        "w_j2lo": np.ascontiguousarray(
            wj2lo.transpose(1, 0, 2, 3).reshape(128, 3 * 2 * V)),
        "biases": biases,
        # psum carries 2^16 * out, so the (rare) nonzero-bias path adds the
        # bias pre-scaled to match
        "b_j2": bj2 * np.float32(SH * SW),
    }
    in_maps = []
    for c in range(NCORES):
        enc_sel = np.zeros((256, E), np.float32)
        for b in range(B):
            ts = c + NCORES * np.arange(G[b])
            valid = ts < ENC_SIZES[b]
            rows = np.where(valid)[0]
            enc_sel[OFF_T[b] + rows] = enc[b, ts[valid]]
        encT = np.ascontiguousarray(enc_sel.T).astype(NP_BF16)   # [512, 256]
        in_maps.append({"encT": _kp(encT, 4, 256), **common})
    return in_maps, zero_bj2


def _gather_output(core_outs, inputs):
    fb = np.asarray(inputs["flat_b"]).astype(np.int64)
    ft = np.asarray(inputs["flat_t"]).astype(np.int64)
    fu = np.asarray(inputs["flat_u"]).astype(np.int64)
    ub1 = np.asarray(UB1, np.int64)
    off_r = np.asarray(OFF_R[:B], np.int64)
    core = ft % NCORES
    local = off_r[fb] + (ft // NCORES) * ub1[fb] + fu
    out = np.empty((fb.shape[0], V), np.float32)
    for c in range(NCORES):
        m = core == c
        out[m] = core_outs[c][local[m]]
    out *= np.float32(OUT_SCALE)
    return out


def kernel(**inputs) -> np.ndarray:
    in_maps, zero_bj2 = _host_inputs(inputs)
    key = ("nc", zero_bj2)
    if key not in _cache:
        _cache[key] = _build(zero_bj2=zero_bj2)
        _cache["nc"] = _cache[key]
    nc = _cache[key]
    res = run_bass_kernel_spmd(nc, in_maps, list(range(NCORES))).results
    core_outs = [res[c]["out"].astype(np.float32) for c in range(NCORES)]
    return _gather_output(core_outs, inputs)


# revision 23
# speedup vs baseline: 1.2531x; 1.0033x over previous
"""RNN-T joiner (nn_CombinationModel_53154515256115) as a Bass/Tile SPMD kernel
for 8 Trainium2 NeuronCores.

Algorithm
---------
The reference computes, for each valid (b, t, u):
    out[b,t,u] = relu(enc[b,t] @ Wj1_enc + pred[b,u] @ Wj1_pred + bj1) @ Wj2 + bj2
The joint pre-activation factors into a per-(b,t) term A and a per-(b,u) term
Pp, collapsing the first joiner matmul from ~95 GFLOP to ~2 GFLOP. The
remaining dominant work is the [N,640] @ [640,1056] output matmul plus the
ragged broadcast-add expansion and the output write.

The output matmul runs in fp8 (e4m3) with DoubleRow perf mode (two
contraction k-tiles per instruction at 0.5 cycles/row) and full
error-feedback so precision stays at bf16 level:
    psum = h8@W8 + dh8@W8 + h8@dW8  (+ 2^16 * b_j2)
with h8 = e4m3(64*h), dh8 = e4m3(64*h - h8), W8 = e4m3(1024*W_j2),
dW8 = e4m3(1024*W_j2 - W8). The psum is therefore 2^16 * out; it is written
as fp16 and rescaled by 2^-16 on the host (exact power of two).
Per 128-row tile that is 8 DoubleRow matmuls (vs 15 bf16 k-tile equivalents).

Weights arrive pre-cast (bf16 / e4m3) and pre-transposed from the host (one
batched DMA per tensor), and the prediction-network embedding gather runs on
the host, so the device does no fp32 weight traffic, no embedding gather and
no PE transposes. The elementwise expansion work (broadcast add, relu/cast,
residual, psum evacuation) is statically load-balanced across Pool, Act and
DVE.

Sharding (SPMD-uniform)
-----------------------
Core c takes encoder frames t with t % 8 == c from every batch. Every core
then runs an identical program shape: per batch b it owns G[b] = ceil(T_b/8)
frame-groups of (U_b+1) rows each (8134 rows/core; rows of garbage frame-
groups where c + 8g >= T_b are dropped on the host). The tiny prediction
network (264 packed rows) is computed replicated on every core.
"""

from contextlib import ExitStack

import numpy as np
import ml_dtypes

import concourse.bass as bass
import concourse.mybir as mybir
import concourse.tile as tile
from concourse import bacc
from concourse.bass_utils import run_bass_kernel_spmd

F32 = mybir.dt.float32
F16 = mybir.dt.float16
BF16 = mybir.dt.bfloat16
F8E4 = mybir.dt.float8e4
AF = mybir.ActivationFunctionType
ALU = mybir.AluOpType
DR = mybir.MatmulPerfMode.DoubleRow

NP_BF16 = ml_dtypes.bfloat16
NP_E4 = ml_dtypes.float8_e4m3

# ---------------------------------------------------------------- constants
B, T, U = 8, 300, 40
E, P, J, V = 512, 640, 640, 1056
H, DEMB = 2, 256
ENC_SIZES = [300, 280, 260, 240, 220, 210, 205, 200]
TGT_SIZES = [40, 38, 35, 33, 30, 28, 26, 25]
NCORES = 8

G = [(t + NCORES - 1) // NCORES for t in ENC_SIZES]       # groups/core/batch
UB1 = [u + 1 for u in TGT_SIZES]                          # u-extent per batch
RBV = [G[b] * UB1[b] for b in range(B)]                   # valid rows/batch
ROWS = sum(RBV)                                           # 8134 rows/core
GT_TOT = sum(G)                                           # 242 enc frames/core
OFF_T = [0]
for b in range(B):
    OFF_T.append(OFF_T[-1] + G[b])
OFF_R = [0]
for b in range(B):
    OFF_R.append(OFF_R[-1] + RBV[b])
UOFF = [0]
for b in range(B):
    UOFF.append(UOFF[-1] + UB1[b])
UTOT = UOFF[-1]               # 263 valid prediction rows
UPAD = 264
# HD plane stride: DoubleRow Ldweights requires the dual-row step to be a
# multiple of 16 elements (s3_lw_dual_fp8_restrictions)
RPAD = (ROWS + 15) // 16 * 16

NJ = J // 128                 # 5 partition tiles of the 640-dim feature axis
V_CHUNKS = [(0, 512), (512, 512), (1024, V - 1024)]
NTILES = (ROWS + 127) // 128

SH = 64.0                     # h-side fp8 scale
SW = 1024.0                   # W-side fp8 scale  (psum = 2^16 * out)
OUT_SCALE = 1.0 / (SH * SW)

# expansion work units: (b, g0, g1); first batches split so the main loop's
# first tiles are not gated on one huge Pool op per plane
_EXP_SPLITS = {0: 3, 1: 2}
EXP_CHUNKS = []
for b in range(B):
    ns = _EXP_SPLITS.get(b, 1)
    bounds = [round(i * G[b] / ns) for i in range(ns + 1)]
    for i in range(ns):
        EXP_CHUNKS.append((b, bounds[i], bounds[i + 1]))

PS_SMALL_BUFS = 2

# ------------- engine-split knobs (tuned against the instruction cost model)
TRICKLE = 1
DEADLINE = 2
HORIZON = 14
ADD_DVE_EVERY = 4      # every n-th expansion add runs on DVE instead of Pool
H8_POOL_EVERY = 10**9  # disabled: Pool is the critical engine
# evac engine rotation: A=Act, D=DVE, P=Pool
EV_PAT = ("A", "D", "A", "D", "A", "A", "D", "A",
          "A", "D", "A", "A", "A", "D", "A", "D")

_cache = {}


def _build(zero_bj2=True):
    nc = bacc.Bacc("TRN2", target_bir_lowering=False, debug=False,
                   num_devices=NCORES)

    # one dram tensor per logical input, pre-laid-out as [128, ...] so each
    # loads with a single wide DMA
    encT_d = nc.dram_tensor("encT", [128, 4 * 256], BF16, kind="ExternalInput").ap()
    eT_d = nc.dram_tensor("eT", [128, 4 * UPAD], BF16, kind="ExternalInput").ap()
    wp1_d = nc.dram_tensor("w_p1", [128, 4 * P], BF16, kind="ExternalInput").ap()
    wp2_d = nc.dram_tensor("w_p2", [128, 5 * P], BF16, kind="ExternalInput").ap()
    wj1e_d = nc.dram_tensor("w_j1e", [128, 4 * J], BF16, kind="ExternalInput").ap()
    wj1p_d = nc.dram_tensor("w_j1p", [128, 5 * J], BF16, kind="ExternalInput").ap()
    wj2hi_d = nc.dram_tensor("w_j2hi", [128, 5 * V], F8E4, kind="ExternalInput").ap()
    wj2lo_d = nc.dram_tensor("w_j2lo", [128, 3 * 2 * V], F8E4,
                             kind="ExternalInput").ap()
    bias_d = nc.dram_tensor("biases", [128, 15], F32, kind="ExternalInput").ap()
    bj2_d = nc.dram_tensor("b_j2", [V], F32, kind="ExternalInput").ap()
    out_d = nc.dram_tensor("out", [ROWS, V], F16, kind="ExternalOutput").ap()

    R = RPAD
    MAXCH = max((g1 - g0) * UB1[b] for (b, g0, g1) in EXP_CHUNKS)

    with tile.TileContext(nc) as tc:
      with ExitStack() as ctx:
        persist = ctx.enter_context(tc.tile_pool(name="persist", bufs=1))
        stage = ctx.enter_context(tc.tile_pool(name="stage", bufs=1))
        tmpp = ctx.enter_context(tc.tile_pool(name="tmpp", bufs=4))
        hfp = ctx.enter_context(tc.tile_pool(name="hfp", bufs=4))
        outp = ctx.enter_context(tc.tile_pool(name="outp", bufs=3))
        ps_small = ctx.enter_context(tc.tile_pool(name="ps_small", bufs=PS_SMALL_BUFS, space="PSUM"))
        ps_main = ctx.enter_context(tc.tile_pool(name="ps_main", bufs=2, space="PSUM"))

        # ---------------- persistent SBUF state
        # HD planes (free dim, ROWS wide each): h0,d0,h1,d1,...,h4,d4
        HD = persist.tile([128, 10 * R], F8E4, tag="hd", name="hd")
        wj2hi_b = persist.tile([128, 5 * V], F8E4, tag="wj2h", name="wj2h")
        wj2lo_b = persist.tile([128, 3 * 2 * V], F8E4, tag="wj2l", name="wj2l")
        at_t = [persist.tile([128, GT_TOT], BF16, tag=f"at_{j}", name=f"at_{j}")
                for j in range(NJ)]
        pp_t = [persist.tile([128, UPAD], BF16, tag=f"pp_{j}", name=f"pp_{j}")
                for j in range(NJ)]

        # ---------------- input loads: one DMA per tensor, two queues,
        # ordered by first use
        eT_b = stage.tile([128, 4 * UPAD], BF16, tag="eT", name="eT")
        wp1_b = stage.tile([128, 4 * P], BF16, tag="wp1", name="wp1")
        wp2_b = stage.tile([128, 5 * P], BF16, tag="wp2", name="wp2")
        encT_b = stage.tile([128, 4 * 256], BF16, tag="encT", name="encT")
        wj1e_b = stage.tile([128, 4 * J], BF16, tag="wj1e", name="wj1e")
        wj1p_b = stage.tile([128, 5 * J], BF16, tag="wj1p", name="wj1p")
        bias_t = stage.tile([128, 15], F32, tag="bias")

        nc.sync.dma_start(out=eT_b[:], in_=eT_d)
        nc.scalar.dma_start(out=wp1_b[:], in_=wp1_d)
        nc.scalar.dma_start(out=bias_t[:], in_=bias_d)
        nc.sync.dma_start(out=encT_b[:], in_=encT_d)
        nc.scalar.dma_start(out=wj1e_b[:], in_=wj1e_d)
        nc.sync.dma_start(out=wp2_b[:], in_=wp2_d)
        nc.scalar.dma_start(out=wj1p_b[:], in_=wj1p_d)
        nc.sync.dma_start(out=wj2hi_b[:], in_=wj2hi_d)
        nc.sync.dma_start(out=wj2lo_b[:], in_=wj2lo_d)
        bp1_t = bias_t[:, 0:5]
        bp2_t = bias_t[:, 5:10]
        bj1_t = bias_t[:, 10:15]

        bt_t = None
        if not zero_bj2:
            bt_t = persist.tile([128, V], F32, tag="bt", name="bt")
            nc.sync.dma_start(out=bt_t[:], in_=bj2_d[None, :].to_broadcast([128, V]))

        # ---------------- prediction network (replicated, 264 packed rows)
        h1_t = [stage.tile([128, UPAD], BF16, tag=f"h1_{j}", name=f"h1_{j}") for j in range(5)]
        pred_t = [stage.tile([128, UPAD], BF16, tag=f"pred_{j}", name=f"pred_{j}")
                  for j in range(5)]

        def a_block(j):
            ps = ps_small.tile([128, UPAD], F32, tag="ps_s", name="ps_at")
            for k in range(4):
                nc.tensor.matmul(out=ps[:, 0:GT_TOT],
                                 lhsT=wj1e_b[:, k * J + j * 128:k * J + (j + 1) * 128],
                                 rhs=encT_b[:, k * 256:k * 256 + GT_TOT],
                                 start=(k == 0), stop=(k == 3))
            nc.scalar.activation(at_t[j][:], ps[:, 0:GT_TOT], AF.Identity,
                                 bias=bj1_t[:, j:j + 1], scale=1.0)

        # stage1 interleaved with the A blocks (A only needs DMAs, so it
        # fills PE gaps while the Act engine runs the stage tanhs)
        for j in range(5):
            ps = ps_small.tile([128, UPAD], F32, tag="ps_s", name="ps_h1")
            for k in range(4):
                nc.tensor.matmul(out=ps[:],
                                 lhsT=wp1_b[:, k * P + j * 128:k * P + (j + 1) * 128],
                                 rhs=eT_b[:, k * UPAD:(k + 1) * UPAD],
                                 start=(k == 0), stop=(k == 3))
            nc.scalar.activation(h1_t[j][:], ps[:], AF.Tanh,
                                 bias=bp1_t[:, j:j + 1], scale=1.0)
            a_block(j)
        for j in range(5):
            ps = ps_small.tile([128, UPAD], F32, tag="ps_s", name="ps_p2")
            for k in range(5):
                nc.tensor.matmul(out=ps[:],
                                 lhsT=wp2_b[:, k * P + j * 128:k * P + (j + 1) * 128],
                                 rhs=h1_t[k][:],
                                 start=(k == 0), stop=(k == 4))
            nc.scalar.activation(pred_t[j][:], ps[:], AF.Tanh,
                                 bias=bp2_t[:, j:j + 1], scale=1.0)

        def s3_block(j):
            ps = ps_small.tile([128, UPAD], F32, tag="ps_s", name="ps_pp")
            for k in range(5):
                nc.tensor.matmul(out=ps[:],
                                 lhsT=wj1p_b[:, k * J + j * 128:k * J + (j + 1) * 128],
                                 rhs=pred_t[k][:],
                                 start=(k == 0), stop=(k == 4))
            nc.vector.tensor_copy(pp_t[j][:], ps[:])

        # ---------------- ragged expansion into fp8 h8/dh8 planes
        exp_ctr = [0, 0]        # add-counter, h8-counter

        def emit_expand(ci, j):
            b, g0, g1 = EXP_CHUNKS[ci]
            gg, u1 = g1 - g0, UB1[b]
            rv = gg * u1
            o = OFF_R[b] + g0 * u1
            tmp = tmpp.tile([128, MAXCH], BF16, tag="tmp", name="tmp")
            # tmp = at[t] + pp[u]   (Pool, every n-th on DVE)
            exp_ctr[0] += 1
            eng = nc.vector if exp_ctr[0] % ADD_DVE_EVERY == 0 else nc.gpsimd
            eng.tensor_tensor(
                out=tmp[:, 0:rv].rearrange("p (g u) -> p g u", g=gg),
                in0=at_t[j][:, OFF_T[b] + g0:OFF_T[b] + g1][:, :, None]
                    .to_broadcast([128, gg, u1]),
                in1=pp_t[j][:, UOFF[b]:UOFF[b] + u1][:, None, :]
                    .to_broadcast([128, gg, u1]),
                op=ALU.add)
            # h8 = e4m3(relu(64*tmp))   (Act, every n-th on Pool)
            exp_ctr[1] += 1
            h8_out = HD[:, (2 * j) * R + o:(2 * j) * R + o + rv]
            if exp_ctr[1] % H8_POOL_EVERY == 0:
                nc.gpsimd.tensor_scalar(out=h8_out, in0=tmp[:, 0:rv],
                                        scalar1=SH, scalar2=0.0,
                                        op0=ALU.mult, op1=ALU.max)
            else:
                nc.scalar.activation(h8_out, tmp[:, 0:rv], AF.Relu, scale=SH)
            # hf = max(64*tmp, 0) in bf16   (DVE, 4x mode)
            hf = hfp.tile([128, MAXCH], BF16, tag="hf", name="hf")
            nc.vector.tensor_scalar(out=hf[:, 0:rv], in0=tmp[:, 0:rv],
                                    scalar1=SH, scalar2=0.0,
                                    op0=ALU.mult, op1=ALU.max)
            # dh8 = e4m3(hf - h8)   (DVE)
            nc.vector.tensor_tensor(
                out=HD[:, (2 * j + 1) * R + o:(2 * j + 1) * R + o + rv],
                in0=hf[:, 0:rv],
                in1=h8_out,
                op=ALU.subtract)

        # s3 + pp evac per j, immediately followed by that j's first-chunk
        # expansion so the b=0 pipeline fills as early as possible
        for j in range(5):
            s3_block(j)
            emit_expand(0, j)

        # ---------------- main loop
        def emit_main_tile(rt):
            m = min(128, ROWS - rt * 128)
            r0 = rt * 128
            ps = ps_main.tile([128, V], F32, tag="ps_out", name="ps_out")

            def lhs_pair(plane, stride2):
                # [128, 2, m] view of HD planes (plane, plane+stride2)
                seg = HD[:, plane * R:(plane + 2 * stride2) * R]
                if stride2 == 1:
                    v = seg.rearrange("p (two r) -> p two r", two=2)
                else:
                    v = seg.rearrange("p (two x r) -> p two x r", two=2, x=2)[:, :, 0]
                return v[:, :, r0:r0 + m]

            mms = []
            for j in range(5):      # (h8_j | dh8_j) x (W8_j | W8_j)
                mms.append((lhs_pair(2 * j, 1), None, j))
            mms.append((lhs_pair(0, 2), 0, None))   # (h8_0|h8_1) x (dW0|dW1)
            mms.append((lhs_pair(4, 2), 1, None))   # (h8_2|h8_3) x (dW2|dW3)
            mms.append((lhs_pair(8, 1), 2, None))   # (h8_4|dh8_4) x (dW4|Z)

            for i, (lhsT, q, j) in enumerate(mms):
                for (c0, cn) in V_CHUNKS:
                    if q is None:
                        rhs = wj2hi_b[:, None, j * V + c0:j * V + c0 + cn] \
                            .to_broadcast([128, 2, cn])
                    else:
                        rhs = wj2lo_b[:, 2 * q * V:2 * (q + 1) * V].rearrange(
                            "p (two v) -> p two v", two=2)[:, :, c0:c0 + cn]
                    nc.tensor.matmul(out=ps[0:m, c0:c0 + cn],
                                     lhsT=lhsT, rhs=rhs,
                                     start=(i == 0), stop=(i == 7),
                                     perf_mode=DR)

            osb = outp.tile([128, V], F16, tag="osb", name="osb")
            if zero_bj2:
                ev = EV_PAT[rt % len(EV_PAT)]
                if ev == "A":
                    nc.scalar.activation(osb[0:m], ps[0:m], AF.Copy)
                elif ev == "D":
                    nc.vector.tensor_copy(osb[0:m], ps[0:m])
                else:
                    nc.gpsimd.tensor_copy(osb[0:m], ps[0:m])
            else:
                nc.vector.tensor_tensor(out=osb[0:m], in0=ps[0:m],
                                        in1=bt_t[0:m], op=ALU.add)
            nc.sync.dma_start(out=out_d[r0:r0 + m, :], in_=osb[0:m])

        exp_q = [(ci, j) for ci in range(1, len(EXP_CHUNKS)) for j in range(NJ)]

        def chunk_start_row(ci):
            b, g0, g1 = EXP_CHUNKS[ci]
            return OFF_R[b] + g0 * UB1[b]

        for rt in range(NTILES):
            while exp_q and chunk_start_row(exp_q[0][0]) < (rt + DEADLINE) * 128:
                emit_expand(*exp_q.pop(0))
            for _ in range(TRICKLE):
                if exp_q and chunk_start_row(exp_q[0][0]) < (rt + HORIZON) * 128:
                    emit_expand(*exp_q.pop(0))
            emit_main_tile(rt)

    nc.compile()
    return nc


def _kp(w, kt, cols):
    """[kt*128, cols] -> [128, kt*cols] (k-tile-major columns)."""
    return np.ascontiguousarray(
        w.reshape(kt, 128, cols).transpose(1, 0, 2).reshape(128, kt * cols))


def _host_inputs(inputs):
    """Build per-core in_maps from the full inputs (cast + transpose + gather
    staging only; all model compute stays on device)."""
    enc = np.ascontiguousarray(np.asarray(inputs["encoder_states"], dtype=np.float32))
    targets = np.asarray(inputs["targets"]).astype(np.int64)
    emb = np.asarray(inputs["emb"], dtype=np.float32)
    wj1 = np.asarray(inputs["W_j1"], dtype=np.float32)
    wj2 = np.asarray(inputs["W_j2"], dtype=np.float32)
    bj2 = np.asarray(inputs["b_j2"], dtype=np.float32)

    # prediction-network input e: H-gram context embeddings, padding_idx=0,
    # packed to the 263 valid (b, u) rows (+1 pad row)
    ext = np.zeros((B, U + H), np.int64)
    ext[:, H:] = targets
    ctx = np.stack([ext[:, H - 1 - i: U + H - i] for i in range(H)], axis=-1)
    e = emb[ctx] * (ctx != 0)[..., None]                 # [B, U+1, H, DEMB]
    e = e.reshape(B, U + 1, H * DEMB)
    e_sel = np.zeros((UPAD, H * DEMB), np.float32)
    for b in range(B):
        e_sel[UOFF[b]:UOFF[b] + UB1[b]] = e[b, :UB1[b]]
    eT = np.ascontiguousarray(e_sel.T).astype(NP_BF16)   # [512, 264]

    # fp8 split of W_j2 (scaled by SW) + zero pair-slot for the bias trick
    w_hi = (SW * wj2).astype(NP_E4)
    w_lo = (SW * wj2 - w_hi.astype(np.float32)).astype(NP_E4)
    zero_bj2 = not np.any(bj2)
    wj2lo = np.zeros((3, 128, 2, V), NP_E4)
    wj2lo[0, :, 0] = w_lo[0:128]
    wj2lo[0, :, 1] = w_lo[128:256]
    wj2lo[1, :, 0] = w_lo[256:384]
    wj2lo[1, :, 1] = w_lo[384:512]
    wj2lo[2, :, 0] = w_lo[512:640]
    # pair slot 2/plane 1 multiplies dh8_4 -> keep zero (bias added via bt
    # tile when bj2 != 0)

    biases = np.zeros((128, 15), np.float32)
    biases[:, 0:5] = np.asarray(inputs["b_pred1"], np.float32).reshape(5, 128).T
    biases[:, 5:10] = np.asarray(inputs["b_pred2"], np.float32).reshape(5, 128).T
    biases[:, 10:15] = np.asarray(inputs["b_j1"], np.float32).reshape(5, 128).T

    common = {
        "eT": _kp(eT, 4, UPAD),
        "w_p1": _kp(np.asarray(inputs["W_pred1"], np.float32).astype(NP_BF16), 4, P),
        "w_p2": _kp(np.asarray(inputs["W_pred2"], np.float32).astype(NP_BF16), 5, P),
        "w_j1e": _kp(wj1[:E].astype(NP_BF16), 4, J),
        "w_j1p": _kp(wj1[E:].astype(NP_BF16), 5, J),
        "w_j2hi": _kp(w_hi, 5, V),
        "w_j2lo": np.ascontiguousarray(
            wj2lo.transpose(1, 0, 2, 3).reshape(128, 3 * 2 * V)),
        "biases": biases,
        # psum carries 2^16 * out, so the (rare) nonzero-bias path adds the
        # bias pre-scaled to match
        "b_j2": bj2 * np.float32(SH * SW),
    }
    in_maps = []
    for c in range(NCORES):
        enc_sel = np.zeros((256, E), np.float32)
        for b in range(B):
            ts = c + NCORES * np.arange(G[b])
            valid = ts < ENC_SIZES[b]
            rows = np.where(valid)[0]
            enc_sel[OFF_T[b] + rows] = enc[b, ts[valid]]
        encT = np.ascontiguousarray(enc_sel.T).astype(NP_BF16)   # [512, 256]
        in_maps.append({"encT": _kp(encT, 4, 256), **common})
    return in_maps, zero_bj2


def _gather_output(core_outs, inputs):
    fb = np.asarray(inputs["flat_b"]).astype(np.int64)
    ft = np.asarray(inputs["flat_t"]).astype(np.int64)
    fu = np.asarray(inputs["flat_u"]).astype(np.int64)
    ub1 = np.asarray(UB1, np.int64)
    off_r = np.asarray(OFF_R[:B], np.int64)
    core = ft % NCORES
    local = off_r[fb] + (ft // NCORES) * ub1[fb] + fu
    out = np.empty((fb.shape[0], V), np.float32)
    for c in range(NCORES):
        m = core == c
        out[m] = core_outs[c][local[m]]
    out *= np.float32(OUT_SCALE)
    return out


def kernel(**inputs) -> np.ndarray:
    in_maps, zero_bj2 = _host_inputs(inputs)
    key = ("nc", zero_bj2)
    if key not in _cache:
        _cache[key] = _build(zero_bj2=zero_bj2)
        _cache["nc"] = _cache[key]
    nc = _cache[key]
    res = run_bass_kernel_spmd(nc, in_maps, list(range(NCORES))).results
    core_outs = [res[c]["out"].astype(np.float32) for c in range(NCORES)]
    return _gather_output(core_outs, inputs)
